# revision 1
# baseline (speedup 1.0000x reference)
"""Expert-parallel MoE GroupedMLP kernel for 8 Trainium2 NeuronCores.

Problem: T=4096 tokens, H=2048 hidden, E=8 experts, I=4096 intermediate,
top_k=2, fp32 reference.

Strategy (expert-parallel, sharded inside kernel()):
  - Host: softmax + top-k routing, all-to-all token dispatch (gather each
    expert's assigned tokens), weight transpose + bf16 cast.  This is the
    sharding/unsharding step; all heavy FLOPs run on device.
  - Device (one expert per core): batched MLP over the expert's gathered
    tokens, padded to capacity C.  bf16 matmuls with fp32 PSUM
    accumulation; SiLU on the scalar engine; combine-weight scaling on the
    vector engine.
  - Host: scatter-add the 8 per-expert outputs into the full [T, H] result.
"""

import time

import numpy as np
import ml_dtypes

from concourse import bass, bacc, tile, mybir
from concourse.bass_utils import run_bass_kernel_spmd

# Problem dims (hardcoded per contract)
T, H, E, I = 4096, 2048, 8, 4096
P = 128          # partitions
KH = H // P      # 16 contraction tiles for MM1
NJ = I // P      # 32 intermediate tiles
HCH = 512        # output hidden chunk
NH = H // HCH    # 4

_BF16 = mybir.dt.bfloat16
_F32 = mybir.dt.float32


def chunk_plan(max_count):
    """Token chunk sizes covering the max per-expert load.  Chunks are the
    matmul moving dim: <=512 (one fp32 PSUM bank), multiples of 128.  At
    most 3 chunks (2*3 MM1 psum banks + 2 MM2 banks = 8); larger loads are
    handled by multiple waves in kernel()."""
    cap = max(P, -(-max_count // P) * P)
    cap = min(cap, 1536)
    plan = [512] * (cap // 512)
    if cap % 512:
        plan.append(cap % 512)
    return tuple(plan)


def build_kernel(plan):
    C = sum(plan)
    nc = bacc.Bacc("TRN2", target_bir_lowering=False, debug=False, num_devices=E)
    xg_d = nc.dram_tensor("xg", [H, C], _BF16, kind="ExternalInput").ap()
    w1t_d = nc.dram_tensor("w1t", [H, 2 * I], _BF16, kind="ExternalInput").ap()
    w2t_d = nc.dram_tensor("w2t", [I, H], _BF16, kind="ExternalInput").ap()
    # combine weights pre-tiled on host: cg[p, q] = weight of token q*128+p
    cg_d = nc.dram_tensor("cg", [P, C // P], _F32, kind="ExternalInput").ap()
    yg_d = nc.dram_tensor("yg", [C, H], _F32, kind="ExternalOutput").ap()

    AF = mybir.ActivationFunctionType

    nchunks = len(plan)
    offs = [sum(plan[:i]) for i in range(nchunks)]
    JG = 2                    # w2 sub-slab j-group
    with tile.TileContext(nc) as tc:
        with (
            tc.tile_pool(name="xp", bufs=1) as xp,
            tc.tile_pool(name="w1p", bufs=2) as w1p,
            tc.tile_pool(name="w2p", bufs=NJ // JG + 2) as w2p,
            tc.tile_pool(name="actp", bufs=1) as actp,
            tc.tile_pool(name="cp", bufs=1) as cp,
            tc.tile_pool(name="sp", bufs=2) as sp,
            tc.tile_pool(name="op", bufs=3) as op,
            tc.tile_pool(name="psA", bufs=1, space="PSUM") as psA,
            tc.tile_pool(name="psB", bufs=2, space="PSUM") as psB,
        ):
            def load_w1_pair(jp):
                n0 = jp * 2 * P
                g = w1p.tile([P, KH, 2 * P], _BF16, tag="w1g",
                             name=f"w1g_{jp}")
                u = w1p.tile([P, KH, 2 * P], _BF16, tag="w1u",
                             name=f"w1u_{jp}")
                nc.sync.dma_start(
                    out=g[:],
                    in_=w1t_d[:, n0:n0 + 2 * P].rearrange(
                        "(k p) n -> p k n", p=P))
                nc.sync.dma_start(
                    out=u[:],
                    in_=w1t_d[:, I + n0:I + n0 + 2 * P].rearrange(
                        "(k p) n -> p k n", p=P))
                return g, u

            # first j-pair's weights BEFORE the bulk xg load: the first
            # matmul needs both, and the DMA queues drain in program order
            gu0 = load_w1_pair(0)

            # gathered tokens, fully resident: one [128, C] tile per h-tile
            xtiles = []
            for k in range(KH):
                xk = xp.tile([P, C], _BF16, tag=f"x{k}")
                nc.sync.dma_start(out=xk[:], in_=xg_d[k * P:(k + 1) * P, :])
                xtiles.append(xk)

            # ---- phase A: h1 = x @ w1.T, act = silu(gate)*up ----
            # j-outer: w1 streamed exactly once; all chunks per PSUM group.
            acts = []
            for jp in range(NJ // 2):
                g, u = gu0 if jp == 0 else load_w1_pair(jp)
                for lj in range(2):
                    j = jp * 2 + lj
                    lsl = slice(lj * P, (lj + 1) * P)
                    pgs = [psA.tile([P, pl], _F32, tag=f"pg{c}",
                                    name=f"pg{c}_{j}")
                           for c, pl in enumerate(plan)]
                    pus = [psA.tile([P, pl], _F32, tag=f"pu{c}",
                                    name=f"pu{c}_{j}")
                           for c, pl in enumerate(plan)]
                    for k in range(KH):
                        for c, pl in enumerate(plan):
                            nc.tensor.matmul(
                                pgs[c][:], g[:, k, lsl],
                                xtiles[k][:, offs[c]:offs[c] + pl],
                                start=(k == 0), stop=(k == KH - 1))
                    for k in range(KH):
                        for c, pl in enumerate(plan):
                            nc.tensor.matmul(
                                pus[c][:], u[:, k, lsl],
                                xtiles[k][:, offs[c]:offs[c] + pl],
                                start=(k == 0), stop=(k == KH - 1))
                    at = actp.tile([P, C], _BF16, tag=f"act{j}")
                    for c, pl in enumerate(plan):
                        st = sp.tile([P, pl], _F32, tag="silu")
                        nc.scalar.activation(st[:], pgs[c][:], AF.Sigmoid)
                        nc.vector.tensor_mul(st[:], st[:], pgs[c][:])
                        nc.vector.tensor_mul(
                            at[:, offs[c]:offs[c] + pl], st[:], pus[c][:])
                    acts.append(at)

            # ---- phase B: y = act @ w2.T, scaled by combine weight ----
            ct = cp.tile([P, C // P], _F32, tag="cg")
            nc.sync.dma_start(out=ct[:], in_=cg_d[:])
            for hc in range(NH):
                w2subs = []
                for jg in range(NJ // JG):
                    w2s = w2p.tile([P, JG, HCH], _BF16, tag="w2")
                    nc.sync.dma_start(
                        out=w2s[:],
                        in_=w2t_d[jg * JG * P:(jg + 1) * JG * P,
                                  hc * HCH:(hc + 1) * HCH].rearrange(
                            "(j p) h -> p j h", p=P))
                    w2subs.append(w2s)
                for tq in range(C // P):
                    po = psB.tile([P, HCH], _F32, tag="po")
                    for j in range(NJ):
                        nc.tensor.matmul(
                            po[:], acts[j][:, tq * P:(tq + 1) * P],
                            w2subs[j // JG][:, j % JG, :],
                            start=(j == 0), stop=(j == NJ - 1))
                    ot = op.tile([P, HCH], _F32, tag="out")
                    nc.vector.tensor_scalar_mul(ot[:], po[:], ct[:, tq:tq + 1])
                    nc.sync.dma_start(
                        out=yg_d[tq * P:(tq + 1) * P, hc * HCH:(hc + 1) * HCH],
                        in_=ot[:])
    nc.compile()
    return nc


_NC_CACHE = {}
LAST_RESULTS = []   # BassKernelResults of each wave of the last kernel() call


def _get_nc(plan):
    if plan not in _NC_CACHE:
        _NC_CACHE[plan] = build_kernel(plan)
    return _NC_CACHE[plan]


def _route(router_logits, top_k):
    """Host routing: stable softmax + top-k (ties broken by lower index,
    matching jax.lax.top_k)."""
    logits = np.asarray(router_logits, dtype=np.float32)
    m = logits.max(axis=-1, keepdims=True)
    p = np.exp(logits - m)
    p /= p.sum(axis=-1, keepdims=True)
    ids = np.argsort(-p, axis=-1, kind="stable")[:, :top_k]   # [T, k]
    gates = np.take_along_axis(p, ids, axis=-1)               # [T, k]
    return ids, gates


def kernel(hidden_states, router_logits, w1, w2, top_k):
    top_k = int(top_k)
    x = np.asarray(hidden_states, dtype=np.float32)
    w1 = np.asarray(w1, dtype=np.float32)
    w2 = np.asarray(w2, dtype=np.float32)
    n_tok, hidden = x.shape
    n_exp = w1.shape[0]
    assert (n_tok, hidden, n_exp) == (T, H, E), "compiled for fixed shapes"

    ids, gates = _route(router_logits, top_k)

    # per-expert token lists
    expert_of = ids.ravel()
    token_of = np.repeat(np.arange(n_tok, dtype=np.int64), top_k)
    gate_of = gates.ravel()
    order = np.argsort(expert_of, kind="stable")
    expert_sorted = expert_of[order]
    token_sorted = token_of[order]
    gate_sorted = gate_of[order]
    counts = np.bincount(expert_sorted, minlength=n_exp)
    starts = np.concatenate([[0], np.cumsum(counts)])

    xT = x.T.astype(ml_dtypes.bfloat16)          # [H, T], contiguous
    w1t = [w1[e].T.astype(ml_dtypes.bfloat16) for e in range(n_exp)]
    w2t = [w2[e].T.astype(ml_dtypes.bfloat16) for e in range(n_exp)]

    plan = chunk_plan(int(counts.max()))
    C = sum(plan)
    nc = _get_nc(plan)
    LAST_RESULTS.clear()
    out = np.zeros((n_tok, hidden), dtype=np.float32)
    done = np.zeros(n_exp, dtype=np.int64)   # tokens dispatched per expert
    while True:
        waves = []
        for e in range(n_exp):
            lo = starts[e] + done[e]
            hi = min(starts[e + 1], lo + C)
            waves.append((lo, hi))
        if all(lo >= hi for lo, hi in waves):
            break
        in_maps = []
        toks_per_e = []
        for e, (lo, hi) in enumerate(waves):
            n_e = hi - lo
            toks = token_sorted[lo:hi]
            toks_per_e.append(toks)
            xg = np.zeros((H, C), dtype=ml_dtypes.bfloat16)
            cg = np.zeros((C,), dtype=np.float32)
            if n_e:
                xg[:, :n_e] = xT[:, toks]
                cg[:n_e] = gate_sorted[lo:hi]
            # pre-tile: cg_t[p, q] = cg[q*128 + p]
            cg = np.ascontiguousarray(cg.reshape(C // P, P).T)
            in_maps.append({"xg": xg, "w1t": w1t[e], "w2t": w2t[e], "cg": cg})
            done[e] += n_e
        try:
            res = run_bass_kernel_spmd(nc, in_maps, list(range(E)))
        except Exception:
            # transient device wedge (e.g. NRT_EXEC_UNIT_UNRECOVERABLE)
            # has been observed to clear on retry
            time.sleep(2)
            res = run_bass_kernel_spmd(nc, in_maps, list(range(E)))
        LAST_RESULTS.append(res)
        for e in range(n_exp):
            toks = toks_per_e[e]
            if len(toks):
                out[toks] += res.results[e]["yg"][:len(toks)]
    return out



# revision 5
# speedup vs baseline: 1.0970x; 1.0970x over previous
"""Tensor-parallel MoE GroupedMLP kernel for 8 Trainium2 NeuronCores.

Problem: T=4096 tokens, H=2048 hidden, E=8 experts, I=4096 intermediate,
top_k=2, fp32 reference.

Strategy (tensor-parallel over intermediate_size, sharded inside kernel()):
  - Host: softmax + top-k routing; gather all routed token columns into one
    expert-sorted [H, 8192] bf16 matrix (same for every core); slice w1/w2
    along the intermediate dim so core r owns columns [r*I/8, (r+1)*I/8) of
    every expert.  Zero padding: matmul token chunks use exact run lengths.
  - Device (identical program on all 8 cores; only DRAM contents differ):
    for each expert-run chunk of <=512 tokens: MM1 (x @ w1_slice.T) with
    tokens as the moving dim, SiLU*up on scalar/vector engines, MM2
    (act @ w2_slice.T) producing the partial y in [h, token] layout,
    downcast to bf16 and DMA out.  Chunks are software-pipelined so the
    tensor engine never waits on the activation step.
  - Host: sum the 8 partial y matrices, scale columns by combine weights,
    scatter-add into the full [T, H] fp32 output.
"""

import time

import numpy as np
import ml_dtypes

from concourse import bass, bacc, tile, mybir
from concourse.bass_utils import run_bass_kernel_spmd

# Problem dims (hardcoded per contract)
T, H, E, I = 4096, 2048, 8, 4096
P = 128
KH = H // P          # 16 contraction tiles for MM1
IS = I // 8          # 512: per-core intermediate slice
NJJ = IS // P        # 4 act j-tiles per core
KJ = IS // P         # 4 contraction tiles for MM2
NHC = H // P         # 16 output h-tiles
CHUNK = 512

_BF16 = mybir.dt.bfloat16
_F32 = mybir.dt.float32


def make_chunks(counts):
    """(expert, col offset, n, new_expert) chunks; exact total, balanced
    sizes <=512 (equal-ish splits keep every matmul's moving dim large)."""
    chunks = []
    off = 0
    for e, c in enumerate(counts):
        c = int(c)
        if c == 0:
            continue
        nch = -(-c // CHUNK)
        base, extra = divmod(c, nch)
        for i in range(nch):
            n = base + (1 if i < extra else 0)
            chunks.append((e, off, n, i == 0))
            off += n
    return chunks


def build_kernel(chunks, tt):
    nc = bacc.Bacc("TRN2", target_bir_lowering=False, debug=False, num_devices=8)
    xg_d = nc.dram_tensor("xg", [H, tt], _BF16, kind="ExternalInput").ap()
    # per expert e: cols [e*2*IS, e*2*IS+IS) = gate slice.T, next IS = up slice.T
    w1t_d = nc.dram_tensor("w1t", [H, E * 2 * IS], _BF16, kind="ExternalInput").ap()
    # per expert e: rows [e*IS, (e+1)*IS) = w2[e].T slice [IS, H]
    w2t_d = nc.dram_tensor("w2t", [E * IS, H], _BF16, kind="ExternalInput").ap()
    yg_d = nc.dram_tensor("yg", [H, tt], _BF16, kind="ExternalOutput").ap()

    AF = mybir.ActivationFunctionType

    with tile.TileContext(nc) as tc:
        with (
            tc.tile_pool(name="xp", bufs=3) as xp,
            tc.tile_pool(name="w1p", bufs=2) as w1p,
            tc.tile_pool(name="w2p", bufs=2) as w2p,
            tc.tile_pool(name="actp", bufs=2) as actp,
            tc.tile_pool(name="sp", bufs=3) as sp,
            tc.tile_pool(name="op", bufs=2) as op,
            tc.tile_pool(name="psA", bufs=2, space="PSUM") as psA,
            tc.tile_pool(name="psB", bufs=2, space="PSUM") as psB,
        ):
            def load_slabs(e):
                w1s = w1p.tile([P, KH, 2 * IS], _BF16, tag="w1", name=f"w1_{e}")
                # split into 4 DMAs to spread across queues
                for q in range(4):
                    c0 = q * (2 * IS // 4)
                    c1 = (q + 1) * (2 * IS // 4)
                    nc.sync.dma_start(
                        out=w1s[:, :, c0:c1],
                        in_=w1t_d[:, e * 2 * IS + c0:e * 2 * IS + c1].rearrange(
                            "(k p) j -> p k j", p=P))
                w2s = w2p.tile([P, KJ, H], _BF16, tag="w2", name=f"w2_{e}")
                for q in range(2):
                    r0 = q * (KJ // 2)
                    r1 = (q + 1) * (KJ // 2)
                    nc.sync.dma_start(
                        out=w2s[:, r0:r1, :],
                        in_=w2t_d[e * IS + r0 * P:e * IS + r1 * P, :].rearrange(
                            "(kj p) h -> p kj h", p=P))
                return w1s, w2s

            def load_x(ci):
                _, off, n, _ = chunks[ci]
                xt = xp.tile([P, KH, CHUNK], _BF16, tag="x", name=f"x_{ci}")
                for q in range(4):
                    k0, k1 = q * 4, (q + 1) * 4
                    nc.sync.dma_start(
                        out=xt[:, k0:k1, :n],
                        in_=xg_d[k0 * P:k1 * P, off:off + n].rearrange(
                            "(k p) n -> p k n", p=P))
                return xt

            nchunks = len(chunks)
            slab_cur = load_slabs(chunks[0][0])
            slab_next = None
            xtiles = {0: load_x(0)}
            if nchunks > 1:
                xtiles[1] = load_x(1)

            pending = None   # (act tile, w2 slab, off, n) awaiting MM2
            for ci, (e, off, n, first) in enumerate(chunks):
                if first and ci > 0:
                    slab_cur = slab_next
                # prefetch the next expert's weights one chunk early
                if ci + 1 < nchunks and chunks[ci + 1][3]:
                    slab_next = load_slabs(chunks[ci + 1][0])
                w1s, w2s = slab_cur
                xt = xtiles.pop(ci)
                if ci + 2 < nchunks:
                    xtiles[ci + 2] = load_x(ci + 2)

                # ---- MM1 + act for chunk ci ----
                at = actp.tile([P, NJJ, CHUNK], _BF16, tag="act", name=f"act_{ci}")
                for jj in range(NJJ):
                    pg = psA.tile([P, CHUNK], _F32, tag="pg", name=f"pg_{ci}_{jj}")
                    pu = psA.tile([P, CHUNK], _F32, tag="pu", name=f"pu_{ci}_{jj}")
                    gsl = slice(jj * P, (jj + 1) * P)
                    usl = slice(IS + jj * P, IS + (jj + 1) * P)
                    for k in range(KH):
                        nc.tensor.matmul(pg[:, :n], w1s[:, k, gsl], xt[:, k, :n],
                                         start=(k == 0), stop=(k == KH - 1))
                    for k in range(KH):
                        nc.tensor.matmul(pu[:, :n], w1s[:, k, usl], xt[:, k, :n],
                                         start=(k == 0), stop=(k == KH - 1))
                    st = sp.tile([P, CHUNK], _F32, tag="silu")
                    nc.scalar.activation(st[:, :n], pg[:, :n], AF.Silu)
                    nc.vector.tensor_mul(at[:, jj, :n], st[:, :n], pu[:, :n])

                # ---- MM2 for the previous chunk (PE stays busy on MM1 above
                # while the act of this chunk is produced) ----
                if pending is not None:
                    emit_mm2(nc, psB, op, yg_d, *pending)
                pending = (at, w2s, off, n)

            emit_mm2(nc, psB, op, yg_d, *pending)
    nc.compile()
    return nc


def emit_mm2(nc, psB, op, yg_d, at, w2s, off, n):
    ot = op.tile([P, NHC, CHUNK], _BF16, tag="out", name=f"out_{off}")
    for hc in range(NHC):
        po = psB.tile([P, CHUNK], _F32, tag="po", name=f"po_{off}_{hc}")
        hsl = slice(hc * P, (hc + 1) * P)
        for kj in range(KJ):
            nc.tensor.matmul(po[:, :n], w2s[:, kj, hsl], at[:, kj, :n],
                             start=(kj == 0), stop=(kj == KJ - 1))
        # alternate engines for the PSUM->SBUF downcast copy
        if hc % 2 == 0:
            nc.scalar.copy(ot[:, hc, :n], po[:, :n])
        else:
            nc.vector.tensor_copy(ot[:, hc, :n], po[:, :n])
    for q in range(4):
        h0, h1 = q * 4, (q + 1) * 4
        nc.sync.dma_start(
            out=yg_d[h0 * P:h1 * P, off:off + n].rearrange(
                "(hc p) n -> p hc n", p=P),
            in_=ot[:, h0:h1, :n])


_NC_CACHE = {}
LAST_RESULTS = []   # BassKernelResults of each wave of the last kernel() call


def _get_nc(chunks, tt):
    key = (tuple(chunks), tt)
    if key not in _NC_CACHE:
        _NC_CACHE[key] = build_kernel(chunks, tt)
    return _NC_CACHE[key]


def _route(router_logits, top_k):
    """Host routing: stable softmax + top-k (ties broken by lower index,
    matching jax.lax.top_k)."""
    logits = np.asarray(router_logits, dtype=np.float32)
    m = logits.max(axis=-1, keepdims=True)
    p = np.exp(logits - m)
    p /= p.sum(axis=-1, keepdims=True)
    ids = np.argsort(-p, axis=-1, kind="stable")[:, :top_k]   # [T, k]
    gates = np.take_along_axis(p, ids, axis=-1)               # [T, k]
    return ids, gates


def kernel(hidden_states, router_logits, w1, w2, top_k):
    top_k = int(top_k)
    x = np.asarray(hidden_states, dtype=np.float32)
    w1 = np.asarray(w1, dtype=np.float32)
    w2 = np.asarray(w2, dtype=np.float32)
    n_tok, hidden = x.shape
    n_exp = w1.shape[0]
    assert (n_tok, hidden, n_exp) == (T, H, E), "compiled for fixed shapes"

    ids, gates = _route(router_logits, top_k)

    # flatten pairs, sort by expert (stable)
    expert_of = ids.ravel()
    token_of = np.repeat(np.arange(n_tok, dtype=np.int64), top_k)
    gate_of = gates.ravel().astype(np.float32)
    order = np.argsort(expert_of, kind="stable")
    token_sorted = token_of[order]
    gate_sorted = gate_of[order]
    counts = np.bincount(expert_of, minlength=n_exp)
    tt = int(counts.sum())

    chunks = make_chunks(counts)
    nc = _get_nc(chunks, tt)

    # gathered tokens, transposed: [H, tt] bf16 (same array for all cores)
    xg = np.ascontiguousarray(x.T)[:, token_sorted].astype(ml_dtypes.bfloat16)

    # per-core weight slices
    in_maps = []
    for r in range(8):
        w1t = np.empty((H, E * 2 * IS), dtype=ml_dtypes.bfloat16)
        w2t = np.empty((E * IS, H), dtype=ml_dtypes.bfloat16)
        gsl = slice(r * IS, (r + 1) * IS)
        usl = slice(I + r * IS, I + (r + 1) * IS)
        for e in range(E):
            w1t[:, e * 2 * IS:e * 2 * IS + IS] = w1[e, gsl, :].T
            w1t[:, e * 2 * IS + IS:(e + 1) * 2 * IS] = w1[e, usl, :].T
            w2t[e * IS:(e + 1) * IS, :] = w2[e, :, gsl].T
        in_maps.append({"xg": xg, "w1t": w1t, "w2t": w2t})

    LAST_RESULTS.clear()
    try:
        res = run_bass_kernel_spmd(nc, in_maps, list(range(8)))
    except Exception:
        # transient device wedge has been observed to clear on retry
        time.sleep(2)
        res = run_bass_kernel_spmd(nc, in_maps, list(range(8)))
    LAST_RESULTS.append(res)

    # host reduce: sum partials, apply combine weights, scatter-add
    ysum = res.results[0]["yg"].astype(np.float32)
    for r in range(1, 8):
        ysum += res.results[r]["yg"].astype(np.float32)
    weighted = (ysum * gate_sorted[None, :]).T          # [tt, H]

    out = np.zeros((n_tok, hidden), dtype=np.float32)
    ord2 = np.argsort(token_sorted, kind="stable")
    for k in range(top_k):     # each token appears exactly top_k times
        sel = ord2[k::top_k]
        if k == 0:
            out[token_sorted[sel]] = weighted[sel]
        else:
            out[token_sorted[sel]] += weighted[sel]
    return out


# revision 14
# speedup vs baseline: 1.1112x; 1.0130x over previous
"""Tensor-parallel MoE GroupedMLP kernel for 8 Trainium2 NeuronCores.

Problem: T=4096 tokens, H=2048 hidden, E=8 experts, I=4096 intermediate,
top_k=2, fp32 reference.

Strategy (tensor-parallel over intermediate_size, sharded inside kernel()):
  - Host: softmax + top-k routing; gather all routed token columns into one
    expert-sorted [H, 8192] bf16 matrix (same for every core); slice w1/w2
    along the intermediate dim so core r owns columns [r*I/8, (r+1)*I/8) of
    every expert.  Zero padding: matmul token chunks use exact run lengths.
  - Device (identical program on all 8 cores; only DRAM contents differ):
    for each expert-run chunk of <=512 tokens: MM1 (x @ w1_slice.T) with
    tokens as the moving dim, SiLU*up on scalar/vector engines, MM2
    (act @ w2_slice.T) producing the partial y in [h, token] layout,
    downcast to bf16 and DMA out.  Chunks are software-pipelined so the
    tensor engine never waits on the activation step.
  - Host: sum the 8 partial y matrices, scale columns by combine weights,
    scatter-add into the full [T, H] fp32 output.
"""

import time

import numpy as np
import ml_dtypes

from concourse import bass, bacc, tile, mybir
from concourse.bass_utils import run_bass_kernel_spmd

# Problem dims (hardcoded per contract)
T, H, E, I = 4096, 2048, 8, 4096
P = 128
KH = H // P          # 16 contraction tiles for MM1
IS = I // 8          # 512: per-core intermediate slice
NJJ = IS // P        # 4 act j-tiles per core
KJ = IS // P         # 4 contraction tiles for MM2
NHC = H // P         # 16 output h-tiles
CHUNK = 512

_BF16 = mybir.dt.bfloat16
_F32 = mybir.dt.float32


def make_chunks(counts):
    """(expert, col offset, n, new_expert) chunks; exact total, balanced
    sizes <=512 (equal-ish splits keep every matmul's moving dim large)."""
    chunks = []
    off = 0
    for e, c in enumerate(counts):
        c = int(c)
        if c == 0:
            continue
        nch = -(-c // CHUNK)
        base, extra = divmod(c, nch)
        for i in range(nch):
            n = base + (1 if i < extra else 0)
            chunks.append((e, off, n, i == 0))
            off += n
    return chunks


def build_kernel(chunks, tt):
    nc = bacc.Bacc("TRN2", target_bir_lowering=False, debug=False, num_devices=8)
    xg_d = nc.dram_tensor("xg", [H, tt], _BF16, kind="ExternalInput").ap()
    # pre-tiled on host: row e*128+p, col k*(2*IS)+j  (j: gate IS then up IS)
    w1t_d = nc.dram_tensor("w1t", [E * P, KH * 2 * IS], _BF16,
                           kind="ExternalInput").ap()
    # pre-tiled on host: row e*128+p, col kj*H+h
    w2t_d = nc.dram_tensor("w2t", [E * P, KJ * H], _BF16,
                           kind="ExternalInput").ap()
    yg_d = nc.dram_tensor("yg", [H, tt], _BF16, kind="ExternalOutput").ap()

    AF = mybir.ActivationFunctionType

    with tile.TileContext(nc) as tc:
        with (
            tc.tile_pool(name="xp", bufs=3) as xp,
            tc.tile_pool(name="w1p", bufs=2) as w1p,
            tc.tile_pool(name="w2p", bufs=2) as w2p,
            tc.tile_pool(name="actp", bufs=2) as actp,
            tc.tile_pool(name="sp", bufs=3) as sp,
            tc.tile_pool(name="op", bufs=2) as op,
            tc.tile_pool(name="psA", bufs=2, space="PSUM") as psA,
            tc.tile_pool(name="psB", bufs=2, space="PSUM") as psB,
        ):
            def load_w1(e, fine=False):
                # 2D tile; col = k*(2*IS) + j.  DRAM side is contiguous per
                # partition, so descriptor generation is cheap.
                w1s = w1p.tile([P, KH * 2 * IS], _BF16, tag="w1", name=f"w1_{e}")
                nq = 4 if fine else 1
                step = KH * 2 * IS // nq
                for q in range(nq):
                    nc.scalar.dma_start(
                        out=w1s[:, q * step:(q + 1) * step],
                        in_=w1t_d[e * P:(e + 1) * P, q * step:(q + 1) * step])
                return w1s

            def load_w2(e):
                w2s = w2p.tile([P, KJ * H], _BF16, tag="w2", name=f"w2_{e}")
                nc.scalar.dma_start(out=w2s[:],
                                    in_=w2t_d[e * P:(e + 1) * P, :])
                return w2s

            def load_x(ci, fine=False):
                _, off, n, _ = chunks[ci]
                xt = xp.tile([P, KH, CHUNK], _BF16, tag="x", name=f"x_{ci}")
                nq, kstep = (8, 2) if fine else (4, 4)
                for q in range(nq):
                    k0, k1 = q * kstep, (q + 1) * kstep
                    nc.sync.dma_start(
                        out=xt[:, k0:k1, :n],
                        in_=xg_d[k0 * P:k1 * P, off:off + n].rearrange(
                            "(k p) n -> p k n", p=P))
                return xt

            nchunks = len(chunks)
            # prologue order matters: pieces needed by the first matmuls
            # first, w2 (needed ~27us in) last
            e0 = chunks[0][0]
            w1s0 = load_w1(e0, fine=True)
            xt0 = load_x(0, fine=True)
            xtiles = {0: xt0}
            if nchunks > 1:
                xtiles[1] = load_x(1)
            slab_cur = (w1s0, load_w2(e0))
            slab_next = None

            pending = None   # (act tile, w2 slab, off, n) awaiting MM2
            for ci, (e, off, n, first) in enumerate(chunks):
                if first and ci > 0:
                    slab_cur = slab_next
                # prefetch the next expert's weights one chunk early
                if ci + 1 < nchunks and chunks[ci + 1][3]:
                    en = chunks[ci + 1][0]
                    slab_next = (load_w1(en), load_w2(en))
                w1s, w2s = slab_cur
                xt = xtiles.pop(ci)
                if ci + 2 < nchunks:
                    xtiles[ci + 2] = load_x(ci + 2)

                # ---- MM1 + act for chunk ci ----
                at = actp.tile([P, NJJ, CHUNK], _BF16, tag="act", name=f"act_{ci}")
                for jj in range(NJJ):
                    pg = psA.tile([P, CHUNK], _F32, tag="pg", name=f"pg_{ci}_{jj}")
                    pu = psA.tile([P, CHUNK], _F32, tag="pu", name=f"pu_{ci}_{jj}")
                    for k in range(KH):
                        g0 = k * 2 * IS + jj * P
                        nc.tensor.matmul(pg[:, :n], w1s[:, g0:g0 + P],
                                         xt[:, k, :n],
                                         start=(k == 0), stop=(k == KH - 1))
                    for k in range(KH):
                        u0 = k * 2 * IS + IS + jj * P
                        nc.tensor.matmul(pu[:, :n], w1s[:, u0:u0 + P],
                                         xt[:, k, :n],
                                         start=(k == 0), stop=(k == KH - 1))
                    st = sp.tile([P, CHUNK], _F32, tag="silu")
                    nc.scalar.activation(st[:, :n], pg[:, :n], AF.Silu)
                    nc.vector.tensor_mul(at[:, jj, :n], st[:, :n], pu[:, :n])

                # ---- MM2 for the previous chunk (PE stays busy on MM1 above
                # while the act of this chunk is produced) ----
                if pending is not None:
                    emit_mm2(nc, psB, op, yg_d, *pending)
                pending = (at, w2s, off, n)

            emit_mm2(nc, psB, op, yg_d, *pending, nq=8)
    nc.compile()
    return nc


def emit_mm2(nc, psB, op, yg_d, at, w2s, off, n, nq=4):
    ot = op.tile([P, NHC, CHUNK], _BF16, tag="out", name=f"out_{off}")
    for hc in range(NHC):
        po = psB.tile([P, CHUNK], _F32, tag="po", name=f"po_{off}_{hc}")
        for kj in range(KJ):
            c0 = kj * H + hc * P
            nc.tensor.matmul(po[:, :n], w2s[:, c0:c0 + P], at[:, kj, :n],
                             start=(kj == 0), stop=(kj == KJ - 1))
        # alternate engines for the PSUM->SBUF downcast copy
        if hc % 2 == 0:
            nc.scalar.copy(ot[:, hc, :n], po[:, :n])
        else:
            nc.vector.tensor_copy(ot[:, hc, :n], po[:, :n])
    step = NHC // nq
    for q in range(nq):
        h0, h1 = q * step, (q + 1) * step
        nc.scalar.dma_start(
            out=yg_d[h0 * P:h1 * P, off:off + n].rearrange(
                "(hc p) n -> p hc n", p=P),
            in_=ot[:, h0:h1, :n])


_NC_CACHE = {}
LAST_RESULTS = []   # BassKernelResults of each wave of the last kernel() call


def _get_nc(chunks, tt):
    key = (tuple(chunks), tt)
    if key not in _NC_CACHE:
        _NC_CACHE[key] = build_kernel(chunks, tt)
    return _NC_CACHE[key]


def _route(router_logits, top_k):
    """Host routing: stable softmax + top-k (ties broken by lower index,
    matching jax.lax.top_k)."""
    logits = np.asarray(router_logits, dtype=np.float32)
    m = logits.max(axis=-1, keepdims=True)
    p = np.exp(logits - m)
    p /= p.sum(axis=-1, keepdims=True)
    ids = np.argsort(-p, axis=-1, kind="stable")[:, :top_k]   # [T, k]
    gates = np.take_along_axis(p, ids, axis=-1)               # [T, k]
    return ids, gates


def kernel(hidden_states, router_logits, w1, w2, top_k):
    top_k = int(top_k)
    x = np.asarray(hidden_states, dtype=np.float32)
    w1 = np.asarray(w1, dtype=np.float32)
    w2 = np.asarray(w2, dtype=np.float32)
    n_tok, hidden = x.shape
    n_exp = w1.shape[0]
    assert (n_tok, hidden, n_exp) == (T, H, E), "compiled for fixed shapes"

    ids, gates = _route(router_logits, top_k)

    # flatten pairs, sort by expert (stable)
    expert_of = ids.ravel()
    token_of = np.repeat(np.arange(n_tok, dtype=np.int64), top_k)
    gate_of = gates.ravel().astype(np.float32)
    order = np.argsort(expert_of, kind="stable")
    token_sorted = token_of[order]
    gate_sorted = gate_of[order]
    counts = np.bincount(expert_of, minlength=n_exp)
    tt = int(counts.sum())

    chunks = make_chunks(counts)
    nc = _get_nc(chunks, tt)

    # gathered tokens, transposed: [H, tt] bf16 (same array for all cores)
    xg = np.ascontiguousarray(x.T)[:, token_sorted].astype(ml_dtypes.bfloat16)

    # per-core weight slices, pre-tiled into the SBUF slab layout:
    # w1t row e*128+p, col k*(2*IS)+j ; w2t row e*128+p, col kj*H+h
    in_maps = []
    for r in range(8):
        w1t = np.empty((E * P, KH * 2 * IS), dtype=ml_dtypes.bfloat16)
        w2t = np.empty((E * P, KJ * H), dtype=ml_dtypes.bfloat16)
        gsl = slice(r * IS, (r + 1) * IS)
        usl = slice(I + r * IS, I + (r + 1) * IS)
        for e in range(E):
            wcols = np.concatenate([w1[e, gsl, :], w1[e, usl, :]], axis=0)
            # W^T [H, 2*IS] -> [p, k, j] -> row-major
            w1t[e * P:(e + 1) * P] = (
                wcols.T.reshape(KH, P, 2 * IS).transpose(1, 0, 2)
                .reshape(P, KH * 2 * IS))
            w2t[e * P:(e + 1) * P] = (
                w2[e, :, gsl].T.reshape(KJ, P, H).transpose(1, 0, 2)
                .reshape(P, KJ * H))
        in_maps.append({"xg": xg, "w1t": w1t, "w2t": w2t})

    LAST_RESULTS.clear()
    try:
        res = run_bass_kernel_spmd(nc, in_maps, list(range(8)))
    except Exception:
        # transient device wedge has been observed to clear on retry
        time.sleep(2)
        res = run_bass_kernel_spmd(nc, in_maps, list(range(8)))
    LAST_RESULTS.append(res)

    # host reduce: sum partials, apply combine weights, scatter-add
    ysum = res.results[0]["yg"].astype(np.float32)
    for r in range(1, 8):
        ysum += res.results[r]["yg"].astype(np.float32)
    weighted = (ysum * gate_sorted[None, :]).T          # [tt, H]

    out = np.zeros((n_tok, hidden), dtype=np.float32)
    ord2 = np.argsort(token_sorted, kind="stable")
    for k in range(top_k):     # each token appears exactly top_k times
        sel = ord2[k::top_k]
        if k == 0:
            out[token_sorted[sel]] = weighted[sel]
        else:
            out[token_sorted[sel]] += weighted[sel]
    return out
